# revision 20
# baseline (speedup 1.0000x reference)
"""Trainium2 Bass kernel for nn_MultiHeadAttention_17386027615012.

MHA variant: softmax over the HEAD axis (dim=1), 1/sqrt(emb) applied AFTER
softmax (folded into proj_w host-side). Softmax-over-heads makes every (q, k)
independent -> shard flattened (batch, seq) query rows over 8 cores
(b = core // 4, 1024-row q-chunk = core % 4).

v5: tensor-parallel K/V projection + AllGather (no 4x redundant compute).
 - Each core projects K^T/V only for its OWN 1024-key quarter (the host
   feeds it that x-slice), bounces the result through internal DRAM and
   AllGathers within its 4-core batch group ([[0..3],[4..7]]). Rank order
   == quarter order, so the gathered buffer is global-k-major and the
   program stays fully SPMD-uniform.
 - Steady state streams K^T/V per 512-key block from the gathered DRAM
   buffer (DMA only - no PE projection, no ACT drain), overlapping the
   collective with Q projection + own-quarter work.
 - PSUM pool = four [128,1024] slots; energy/exp per head; Ot(prev) as
   head-pair units at kc4 1; Ot partials accumulate into fp16 ot_sb via
   DVE tensor_add; softmax ta/tb pairwise adds on GpSimd.
"""
import sys

sys.path.insert(0, "/opt/trn_rl_repo")

import numpy as np
import ml_dtypes
from contextlib import ExitStack

import concourse.bass as bass
import concourse.tile as tile
from concourse import bacc, mybir
from concourse import bass_utils

F32 = mybir.dt.float32
BF16 = mybir.dt.bfloat16
FP16 = mybir.dt.float16
EXPF = mybir.ActivationFunctionType.Exp
IDENT = mybir.ActivationFunctionType.Identity

B, N, E, H, D = 2, 4096, 768, 8, 96
NCORES = 4 * B
QC = N // 4          # 1024 q rows per core; also the owned k-quarter size
KB = 512             # k-block (streaming granularity)
NKB = N // KB        # 8
SEG = 4              # 128-key chunks per k-block
NE = E // 128        # 6
GROUPS = [[0, 1, 2, 3], [4, 5, 6, 7]]


def build(use_bias: bool):
    nc = bacc.Bacc("TRN2", debug=False)
    xtq = nc.dram_tensor("xtq", (E, QC), BF16, kind="ExternalInput").ap()
    xkq = nc.dram_tensor("xkq", (E, QC), BF16, kind="ExternalInput").ap()
    wq = nc.dram_tensor("wq", (E, E), BF16, kind="ExternalInput").ap()
    wk = nc.dram_tensor("wk", (E, E), BF16, kind="ExternalInput").ap()
    wv = nc.dram_tensor("wv", (E, E), BF16, kind="ExternalInput").ap()
    pw = nc.dram_tensor("pw", (E, E), FP16, kind="ExternalInput").ap()
    bqk = nc.dram_tensor("bqk", (2, H, D), F32, kind="ExternalInput").ap()
    bv = nc.dram_tensor("bv", (1, E), BF16, kind="ExternalInput").ap()
    out = nc.dram_tensor("out", (QC, E), F32, kind="ExternalOutput").ap()

    with tile.TileContext(nc) as tc, ExitStack() as ctx:
        # ---- persistent pools ----
        wpool = ctx.enter_context(tc.tile_pool(name="wpool", bufs=1))
        qsl = wpool.tile([D, H * QC], BF16, name="qsl")
        ot_sb = wpool.tile([D, H * QC], FP16, name="ot_sb")
        if use_bias:
            bqk_t = wpool.tile([D, 2 * H], F32, name="bqk_t")
            nc.sync.dma_start(
                bqk_t.rearrange("d (c h) -> d c h", c=2),
                bqk.rearrange("c h d -> d c h"),
            )
            ones_t = wpool.tile([1, 128], BF16, name="ones_t")
            nc.vector.memset(ones_t[:], 1.0)
            bv_t = wpool.tile([1, E], BF16, name="bv_t")
            nc.sync.dma_start(bv_t[:], bv[:, :])

        dram = ctx.enter_context(tc.tile_pool(name="dram", bufs=1,
                                              space="DRAM"))
        # own-quarter contributions and gathered (group-wide) K^T / V
        cc_in_k = dram.tile([2, D, H * KB], BF16)
        cc_out_k = dram.tile([NKB, D, H * KB], BF16)
        cc_in_v = dram.tile([2, SEG, 128, E], BF16)
        cc_out_v = dram.tile([NKB, SEG, 128, E], BF16)

        # shared PSUM pool: four [128,1024] slots (2 banks each)
        ppsum = ctx.enter_context(
            tc.tile_pool(name="ppsum", bufs=4, space="PSUM"))

        def pslot():
            return ppsum.tile([128, 1024], F32, name="ps", tag="ps")

        # ---- phase 0: own-quarter K/V projection + AllGather, Q proj ----
        with ExitStack() as actx:
            qppool = actx.enter_context(tc.tile_pool(name="qppool", bufs=1))
            xks = []
            for e in range(NE):
                xk_t = qppool.tile([128, QC], BF16, name=f"xk{e}")
                nc.sync.dma_start(xk_t[:], xkq[e * 128:(e + 1) * 128, :])
                xks.append(xk_t)
            wks = []
            for e in range(NE):
                wk_t = qppool.tile([128, E], BF16, name=f"wk{e}")
                nc.sync.dma_start(wk_t[:], wk[e * 128:(e + 1) * 128, :])
                wks.append(wk_t)
            wvs = []
            for e in range(NE):
                wv_t = qppool.tile([128, E], BF16, name=f"wv{e}")
                nc.sync.dma_start(wv_t[:], wv[e * 128:(e + 1) * 128, :])
                wvs.append(wv_t)
            xqs = []
            for e in range(NE):
                xq_t = qppool.tile([128, QC], BF16, name=f"xq{e}")
                nc.sync.dma_start(xq_t[:], xtq[e * 128:(e + 1) * 128, :])
                xqs.append(xq_t)
            wqs = []
            for e in range(NE):
                wq_t = qppool.tile([128, E], BF16, name=f"wq{e}")
                nc.sync.dma_start(wq_t[:], wq[e * 128:(e + 1) * 128, :])
                wqs.append(wq_t)

            # K^T for the own quarter: [D, H*KB] per half, 2 heads per slot
            for kb2 in range(2):
                ktt_own = qppool.tile([D, H * KB], BF16, name=f"ko{kb2}")
                for hq in range(4):
                    mp = pslot()
                    for hh in range(2):
                        h = 2 * hq + hh
                        for e in range(NE):
                            nc.tensor.matmul(
                                mp[0:D, hh * KB:(hh + 1) * KB],
                                wks[e][:, h * D:(h + 1) * D],
                                xks[e][:, kb2 * KB:(kb2 + 1) * KB],
                                start=(e == 0), stop=(e == NE - 1),
                            )
                    if use_bias:
                        for hh in range(2):
                            h = 2 * hq + hh
                            nc.scalar.activation(
                                ktt_own[:, h * KB:(h + 1) * KB],
                                mp[0:D, hh * KB:(hh + 1) * KB],
                                IDENT, bias=bqk_t[:, H + h:H + h + 1],
                            )
                    else:
                        nc.scalar.copy(
                            ktt_own[:, 2 * hq * KB:(2 * hq + 2) * KB],
                            mp[0:D, :])
                nc.sync.dma_start(cc_in_k[kb2], ktt_own[:])
            nc.gpsimd.collective_compute(
                "AllGather", mybir.AluOpType.bypass,
                replica_groups=GROUPS,
                ins=[cc_in_k.opt()], outs=[cc_out_k.opt()],
            )

            # V for the own quarter: [128, E] per 128-key chunk
            for kb2 in range(2):
                for kc4 in range(SEG):
                    mp = pslot()
                    xcol = kb2 * KB + kc4 * 128
                    for e in range(NE):
                        nc.tensor.matmul(
                            mp[:, 0:512],
                            xks[e][:, xcol:xcol + 128],
                            wvs[e][:, 0:512],
                            start=(e == 0), stop=(e == NE - 1),
                        )
                        nc.tensor.matmul(
                            mp[:, 512:E],
                            xks[e][:, xcol:xcol + 128],
                            wvs[e][:, 512:E],
                            start=(e == 0), stop=(e == NE - 1),
                        )
                    if use_bias:
                        nc.tensor.matmul(
                            mp[:, 0:512], ones_t[:, 0:128], bv_t[:, 0:512],
                            start=False, stop=True, skip_group_check=True,
                        )
                        nc.tensor.matmul(
                            mp[:, 512:E], ones_t[:, 0:128], bv_t[:, 512:E],
                            start=False, stop=True, skip_group_check=True,
                        )
                    vo = qppool.tile([128, E], BF16, name="vo", tag="vo",
                                     bufs=3)
                    nc.scalar.copy(vo[:], mp[:, 0:E])
                    nc.sync.dma_start(cc_in_v[kb2, kc4], vo[:])
            nc.gpsimd.collective_compute(
                "AllGather", mybir.AluOpType.bypass,
                replica_groups=GROUPS,
                ins=[cc_in_v.opt()], outs=[cc_out_v.opt()],
            )

            # Q projection (1 head per slot)
            for h in range(H):
                qp = pslot()
                for i in range(2):
                    for e in range(NE):
                        nc.tensor.matmul(
                            qp[0:D, i * 512:(i + 1) * 512],
                            wqs[e][:, h * D:(h + 1) * D],
                            xqs[e][:, i * 512:(i + 1) * 512],
                            start=(e == 0), stop=(e == NE - 1),
                        )
                if use_bias:
                    nc.scalar.activation(
                        qsl[:, h * QC:(h + 1) * QC], qp[0:D, :],
                        IDENT, bias=bqk_t[:, h:h + 1],
                    )
                else:
                    nc.scalar.copy(qsl[:, h * QC:(h + 1) * QC], qp[0:D, :])

        # ---- main fused loop over k-blocks (K^T/V streamed from DRAM) ----
        with ExitStack() as bctx:
            ktpool = bctx.enter_context(tc.tile_pool(name="ktpool", bufs=1))
            vtpool = bctx.enter_context(tc.tile_pool(name="vtpool", bufs=1))
            expool = bctx.enter_context(tc.tile_pool(name="expool", bufs=1))
            atpool = bctx.enter_context(tc.tile_pool(name="atpool", bufs=1))
            spool = bctx.enter_context(tc.tile_pool(name="spool", bufs=1))

            def fetch_k(kb):
                ktt = ktpool.tile([D, H * KB], BF16, name="ktt", tag="ktt",
                                  bufs=2)
                nc.sync.dma_start(ktt[:], cc_out_k[kb])
                return ktt

            def fetch_v(kb):
                vps = []
                for kc4 in range(SEG):
                    v_t = vtpool.tile([128, E], BF16, name="vt", tag="vt",
                                      bufs=2 * SEG)
                    nc.sync.dma_start(v_t[:], cc_out_v[kb, kc4])
                    vps.append(v_t)
                return vps

            def energy_unit(ktt, kc4, h, exa):
                ep = pslot()
                for i in range(2):
                    nc.tensor.matmul(
                        ep[:, i * 512:(i + 1) * 512],
                        ktt[:, h * KB + kc4 * 128:h * KB + (kc4 + 1) * 128],
                        qsl[:, h * QC + i * 512:h * QC + (i + 1) * 512],
                        start=True, stop=True,
                    )
                nc.scalar.activation(
                    exa[:, h * QC:(h + 1) * QC], ep[:], EXPF)

            def softmax_unit(exa):
                ta = spool.tile([128, 2 * QC], BF16, name="ta", tag="ta",
                                bufs=2)
                nc.gpsimd.tensor_add(
                    ta[:], exa[:, 0:2 * QC], exa[:, 2 * QC:4 * QC])
                tb = spool.tile([128, 2 * QC], BF16, name="tb", tag="tb",
                                bufs=2)
                nc.gpsimd.tensor_add(
                    tb[:], exa[:, 4 * QC:6 * QC], exa[:, 6 * QC:8 * QC])
                nc.vector.tensor_add(ta[:], ta[:], tb[:])
                s32 = spool.tile([128, QC], F32, name="s32", tag="s32",
                                 bufs=1)
                nc.vector.tensor_add(s32[:], ta[:, 0:QC], ta[:, QC:2 * QC])
                nc.vector.reciprocal_approx_fast(s32[:], s32[:])
                r16 = spool.tile([128, QC], BF16, name="r16", tag="r16",
                                 bufs=2)
                nc.vector.tensor_scalar_min(r16[:], s32[:], 3e38)
                att = atpool.tile([128, H * QC], BF16, name="att",
                                  tag="att", bufs=SEG + 1)
                for hp in range(4):
                    sl = slice(2 * hp * QC, (2 * hp + 2) * QC)
                    nc.vector.tensor_mul(
                        att[:, sl].rearrange("p (h q) -> p h q", h=2),
                        exa[:, sl].rearrange("p (h q) -> p h q", h=2),
                        r16[:, None, :].to_broadcast((128, 2, QC)),
                    )
                return att

            def ot_unit(kb_prev, vps, atts, hp):
                ops = [pslot(), pslot()]
                for hh in range(2):
                    h = 2 * hp + hh
                    for i in range(2):
                        cols = slice(i * 512, (i + 1) * 512)
                        qcols = slice(h * QC + i * 512,
                                      h * QC + (i + 1) * 512)
                        for kc4 in range(SEG):
                            nc.tensor.matmul(
                                ops[hh][0:D, cols],
                                vps[kc4][:, h * D:(h + 1) * D],
                                atts[kc4][:, qcols],
                                start=(kc4 == 0), stop=(kc4 == SEG - 1),
                            )
                for hh in range(2):
                    h = 2 * hp + hh
                    dst = ot_sb[:, h * QC:(h + 1) * QC]
                    if kb_prev == 0:
                        nc.vector.tensor_copy(dst, ops[hh][0:D, :])
                    else:
                        nc.vector.tensor_add(dst, dst, ops[hh][0:D, :])

            # ---- software-pipelined block schedule ----
            ktt = fetch_k(0)
            vps = fetch_v(0)
            ktts = {}
            vpss = {}
            prev = None  # (kb, vps, atts) awaiting Ot
            for kb in range(NKB):
                if kb + 1 < NKB:
                    ktts[kb + 1] = fetch_k(kb + 1)
                    vpss[kb + 1] = fetch_v(kb + 1)
                atts = []
                # Ot(prev) head pairs at kc4 1 so PE never head-blocks on
                # the previous kc4-3 softmax chain at the kb boundary.
                for kc4 in range(SEG):
                    exa = expool.tile([128, H * QC], BF16, name="exa",
                                      tag="exa", bufs=2)
                    for h in range(H):
                        energy_unit(ktt, kc4, h, exa)
                        if kc4 == 1 and h % 2 == 1 and prev is not None:
                            ot_unit(prev[0], prev[1], prev[2], h // 2)
                    atts.append(softmax_unit(exa))
                prev = (kb, vps, atts)
                ktt = ktts.pop(kb + 1, None)
                vps = vpss.pop(kb + 1, None)
            # trailing Ot for the last block
            for hp in range(4):
                ot_unit(prev[0], prev[1], prev[2], hp)

        # ---- output projection ----
        with ExitStack() as cctx:
            ppool = cctx.enter_context(tc.tile_pool(name="ppool", bufs=1))
            ostp = cctx.enter_context(tc.tile_pool(name="ostp", bufs=2))
            pws = []
            for h in range(H):
                pw_t = ppool.tile([D, E], FP16, name=f"pw{h}")
                nc.sync.dma_start(pw_t[:], pw[h * D:(h + 1) * D, :])
                pws.append(pw_t)
            for qb in range(QC // 128):
                po = pslot()
                for h in range(H):
                    lhs = ot_sb[:, h * QC + qb * 128:h * QC + (qb + 1) * 128]
                    nc.tensor.matmul(
                        po[:, 0:512], lhs, pws[h][:, 0:512],
                        start=(h == 0), stop=(h == H - 1))
                    nc.tensor.matmul(
                        po[:, 512:E], lhs, pws[h][:, 512:E],
                        start=(h == 0), stop=(h == H - 1))
                ost = ostp.tile([128, E], F32, name="ost")
                nc.scalar.copy(ost[:], po[:, 0:E])
                nc.sync.dma_start(out[qb * 128:(qb + 1) * 128, :], ost[:])

    nc.compile()
    return nc


_CACHE = {}


def _get_program(use_bias: bool):
    if use_bias not in _CACHE:
        _CACHE[use_bias] = build(use_bias)
    return _CACHE[use_bias]


def _prep_inputs(x, qkv_w, qkv_b, proj_w):
    bf = ml_dtypes.bfloat16
    qw = np.ascontiguousarray(qkv_w.reshape(E, H, D, 3))
    wq = np.ascontiguousarray(qw[..., 0].reshape(E, E)).astype(bf)
    wk = np.ascontiguousarray(qw[..., 1].reshape(E, E)).astype(bf)
    wv = np.ascontiguousarray(qw[..., 2].reshape(E, E)).astype(bf)
    pw = np.ascontiguousarray(
        proj_w / np.sqrt(np.float32(E))).astype(np.float16)
    qb = qkv_b.reshape(H, D, 3)
    bqk = np.ascontiguousarray(
        np.stack([qb[..., 0], qb[..., 1]], axis=0)).astype(np.float32)
    bv = np.ascontiguousarray(qb[..., 2].reshape(1, E)).astype(bf)
    xts = [np.ascontiguousarray(x[b].T).astype(bf) for b in range(B)]
    in_maps = []
    for c in range(NCORES):
        b, qi = c // 4, c % 4
        sl = np.ascontiguousarray(xts[b][:, qi * QC:(qi + 1) * QC])
        in_maps.append({
            "xtq": sl,
            "xkq": sl,
            "wq": wq, "wk": wk, "wv": wv, "pw": pw,
            "bqk": bqk, "bv": bv,
        })
    return in_maps


def kernel(x, qkv_w, qkv_b, proj_w, proj_b, _trace=False):
    x = np.asarray(x, dtype=np.float32)
    qkv_w = np.asarray(qkv_w, dtype=np.float32)
    qkv_b = np.asarray(qkv_b, dtype=np.float32)
    proj_w = np.asarray(proj_w, dtype=np.float32)
    proj_b = np.asarray(proj_b, dtype=np.float32)

    use_bias = bool(np.any(qkv_b))
    nc = _get_program(use_bias)
    in_maps = _prep_inputs(x, qkv_w, qkv_b, proj_w)
    res = bass_utils.run_bass_kernel_spmd(
        nc, in_maps, core_ids=list(range(NCORES)), trace=_trace)
    outf = np.empty((B, N, E), dtype=np.float32)
    for c in range(NCORES):
        b, qi = c // 4, c % 4
        outf[b, qi * QC:(qi + 1) * QC, :] = res.results[c]["out"]
    if np.any(proj_b):
        outf += proj_b[None, None, :]
    if _trace:
        kernel.last_exec_time_ns = res.exec_time_ns
        kernel.last_results = res
    return outf
